# revision 1
# baseline (speedup 1.0000x reference)
"""Trainium2 Bass kernel for the soft-LUT cellular-ASIC module.

Math per layer:  state'[b,hw] = clip( sum_p tw[l,p,hw] * prod_m f(c_m, bit_m(p)) )
where c_m[b,hw] = state[b,(h+i)%32,(w-1+j)%32]  (m = i*3+j),  f(c,0)=1-c, f(c,1)=c,
tw = sigmoid(toggle_gates).  bit_m(p) = bit (8-m) of p, i.e. m=0 is the MSB.

Implementation: sum-factorization ("soft-LUT tree").  Per 128-position tile,
A0 = sigmoid(tgT) laid out [128 positions, 512 combos]; 9 lerp steps
A'[q] = A[q] + c_m * (A[q+S] - A[q]) halve the free dim 512 -> 1.  LUT weights
sum to 1 (convex), so clip is a numerical no-op (kept as one cheap op).

Position layout ("PM"): tile t = b*8+th holds 128 positions p = ph*32+w with
h = th*4+ph; so hw = th*128 + p and state lives in SBUF as [128, 16].
Window gathers go through a DRAM array G[b, phx(6), wc(34), th(8)] --
G[b,phx,wc,th] = state[b, (th*4+phx)%32, (wc-1)%32] -- built with duplicated
row-phase (phx 4..5) and wrapped columns so that both the G-build DMAs (from
the [128,16] state tile) and the 9 window gathers G[b, i:i+4, j:j+32, :] are
affine <=3-dim DMAs with contiguous inner dims.

Sharding: data-parallel over batch B=16 across 8 cores (B_local=2, no comms).
"""

import numpy as np

import concourse.bass as bass
import concourse.bacc as bacc
import concourse.mybir as mybir
from concourse import tile
from concourse.bass_utils import run_bass_kernel_spmd

F32 = mybir.dt.float32
AF = mybir.ActivationFunctionType
OP = mybir.AluOpType

L = 4          # layers
NPOS = 512     # 2^9 LUT combos
HW = 1024      # 32*32 grid
BLOC = 2       # batch per core (16 / 8 cores)
NT = 16        # position tiles of 128 (BLOC*HW/128)
NCORES = 8

_CACHE = {}


def _emit_g_build(nc, G, sp, b):
    """Write PM-halo G[b] from state tile sp [128,16] (tiles b*8..b*8+7)."""
    bb = b * 8
    # main: phx 0..3, wc 1..32  <- state rows th*4+phx, col wc-1
    nc.sync.dma_start(out=G[b, 0:4, 1:33, :], in_=sp[:, bb : bb + 8])
    # phx 4..5 = rows (th+1)*4 + e, th 0..6
    nc.sync.dma_start(out=G[b, 4:6, 1:33, 0:7], in_=sp[0:64, bb + 1 : bb + 8])
    # phx 4..5, th=7: rows 32,33 wrap to rows 0,1
    nc.sync.dma_start(out=G[b, 4:6, 1:33, 7:8], in_=sp[0:64, bb : bb + 1])
    # wc=0 <- col 31
    nc.sync.dma_start(out=G[b, 0:4, 0, :], in_=sp[31:128:32, bb : bb + 8])
    nc.sync.dma_start(out=G[b, 4:6, 0, 0:7], in_=sp[31:64:32, bb + 1 : bb + 8])
    nc.sync.dma_start(out=G[b, 4:6, 0, 7:8], in_=sp[31:64:32, bb : bb + 1])
    # wc=33 <- col 0
    nc.sync.dma_start(out=G[b, 0:4, 33, :], in_=sp[0:128:32, bb : bb + 8])
    nc.sync.dma_start(out=G[b, 4:6, 33, 0:7], in_=sp[0:64:32, bb + 1 : bb + 8])
    nc.sync.dma_start(out=G[b, 4:6, 33, 7:8], in_=sp[0:64:32, bb : bb + 1])


def _build():
    nc = bacc.Bacc("TRN2", target_bir_lowering=False, debug=True)

    g0 = nc.declare_dram_parameter("g0", [BLOC, 6, 34, 8], F32, isOutput=False)
    tgt = nc.declare_dram_parameter("tgt", [L, HW, NPOS], F32, isOutput=False)
    out = nc.declare_dram_parameter("out", [128, NT], F32, isOutput=True)

    with tile.TileContext(nc) as tc:
        with (
            tc.tile_pool(name="dram", bufs=2, space="DRAM") as dram,
            tc.tile_pool(name="tg", bufs=4) as tgp,
            tc.tile_pool(name="a0", bufs=10) as a0p,
            tc.tile_pool(name="conv", bufs=2) as convp,
            tc.tile_pool(name="st", bufs=2) as stp,
            tc.tile_pool(name="wk", bufs=4) as wk,
        ):
            state = None  # [128, 16] PM layout
            for l in range(L):
                if l == 0:
                    G = g0
                else:
                    G = dram.tile([BLOC, 6, 34, 8], F32)
                    for b in range(BLOC):
                        _emit_g_build(nc, G, state_d, b)

                # ---- gather conv scalars: conv[p, m*16 + b*8 + th] = c_m
                conv = convp.tile([128, 9 * NT], F32)
                for i in range(3):
                    for j in range(3):
                        m = i * 3 + j
                        for b in range(BLOC):
                            nc.sync.dma_start(
                                out=conv[:, m * NT + b * 8 : m * NT + b * 8 + 8],
                                in_=G[b, i : i + 4, j : j + 32, :],
                            )

                # ---- A0 = sigmoid(tgT) per hw-block (shared by both b)
                a0 = []
                for hb in range(8):
                    tgsb = tgp.tile([128, NPOS], F32)
                    nc.sync.dma_start(
                        out=tgsb[:, :], in_=tgt[l, hb * 128 : (hb + 1) * 128, :]
                    )
                    a0t = a0p.tile([128, NPOS], F32)
                    nc.scalar.activation(a0t[:, :], tgsb[:, :], AF.Sigmoid)
                    a0.append(a0t)

                # ---- contraction tree per position tile
                newstate = stp.tile([128, NT], F32)
                for t in range(NT):
                    cur = a0[t % 8][:, :]
                    for s in range(9):
                        S = 256 >> s
                        c = conv[:, s * NT + t : s * NT + t + 1]
                        d = wk.tile([128, S], F32, tag=f"d{S}")
                        nc.vector.tensor_sub(d[:, :], cur[:, S : 2 * S], cur[:, 0:S])
                        if s == 8:
                            nxt_ap = newstate[:, t : t + 1]
                        else:
                            nxt = wk.tile([128, S], F32, tag=f"a{S}")
                            nxt_ap = nxt[:, :]
                        nc.vector.scalar_tensor_tensor(
                            nxt_ap, d[:, :], c, cur[:, 0:S], OP.mult, OP.add
                        )
                        cur = nxt_ap
                # clip to [0,1] (convexity makes this a numerical no-op; cheap safety)
                nc.vector.tensor_scalar(
                    newstate[:, :], newstate[:, :], 0.0, 1.0, OP.max, OP.min
                )
                state = newstate
                if l < L - 1:
                    # state to DRAM; G is then built DRAM->DRAM (clean deps)
                    state_d = dram.tile([128, NT], F32, tag="state_d")
                    nc.sync.dma_start(out=state_d[:, :], in_=newstate[:, :])

            # ---- write out in PM layout; host unpermutes
            nc.sync.dma_start(out=out[:, :], in_=state[:, :])

    nc.finalize()
    return nc


def _host_g0(xb):
    """Build PM-halo G for the initial state (xb: [2,32,32])."""
    phx = np.arange(6)
    th = np.arange(8)
    hr = (4 * th[None, :] + phx[:, None]) % 32          # [6, 8]
    wc = (np.arange(34) - 1) % 32                       # [34]
    return np.ascontiguousarray(
        xb[:, hr[:, None, :], wc[None, :, None]], dtype=np.float32
    )  # [2, 6, 34, 8]


def _run(x, toggle_gates, trace=False):
    if "nc" not in _CACHE:
        _CACHE["nc"] = _build()
    nc = _CACHE["nc"]

    x = np.asarray(x, dtype=np.float32)
    tg = np.asarray(toggle_gates, dtype=np.float32)
    tgT = np.ascontiguousarray(tg.reshape(L, NPOS, HW).transpose(0, 2, 1))
    in_maps = []
    for c in range(NCORES):
        xb = x[BLOC * c : BLOC * (c + 1)]
        in_maps.append({"g0": _host_g0(xb), "tgt": tgT})

    res = run_bass_kernel_spmd(nc, in_maps, core_ids=list(range(NCORES)), trace=trace)
    outs = []
    for c in range(NCORES):
        pm = np.asarray(res.results[c]["out"])  # [128, 16]
        for b in range(BLOC):
            outs.append(pm[:, b * 8 : (b + 1) * 8].T.reshape(32, 32))
    full = np.stack(outs, axis=0)
    return full, res


def kernel(x, toggle_gates):
    full, _ = _run(x, toggle_gates, trace=False)
    return full



# revision 2
# speedup vs baseline: 3.8730x; 3.8730x over previous
"""Trainium2 Bass kernel for the soft-LUT cellular-ASIC module (fast path).

Math per layer:  state'[b,hw] = clip( sum_p tw[l,p,hw] * prod_m f(c_m, bit_m(p)) )
where c_m[b,hw] = state[b,(h+i)%32,(w+j-1)%32]  (m = i*3+j),  f(c,0)=1-c, f(c,1)=c,
tw = sigmoid(toggle_gates).  bit_m(p) = bit (8-m) of p (m=0 is the MSB).

Implementation: 9-level lerp tree ("soft-LUT contraction"), evaluated in f16
batched across all 16 position tiles at once.  Layout: partition p = ph*32+w
(ph = h%4, w), tile lane t = b*8+th (th = h//4).  The tree state for an engine
slice lives as A[:, q*nt + b*TH + th] (q = remaining LUT combos, th-minor), so
each level is 3 large tensor_tensor ops:
    d = A_hi - A_lo ; e = d * c_bcast ; A' = e + A_lo
with c_bcast a stride-0 broadcast AP over q (keeps the f16 2x DVE mode: the
cost model only requires the innermost AP dim packed).

Window gathers are done on-chip: h-rolls are quadrant-aligned partition copies
(+ th-shift pieces for ph wrap), w-rolls are stream_shuffle ops (within-32
partition permutation).  No DRAM round-trip between layers.

Engine split: DVE owns th 0..4 (10 of 16 lanes), Pool (gpsimd) owns th 5..7;
the two tree chains are fully independent per layer and only join at the
[128,16] state tile.  Act does the sigmoids (strided interleaved writes),
prefetched one layer ahead; toggle gates stream in as f16, one layer per DMA.

Sharding: data-parallel over batch B=16 across 8 cores (B_local=2, no comms).
"""

import numpy as np

import concourse.bass as bass
import concourse.bacc as bacc
import concourse.mybir as mybir
from concourse import tile
from concourse.bass_utils import run_bass_kernel_spmd

F32 = mybir.dt.float32
F16 = mybir.dt.float16
AF = mybir.ActivationFunctionType
OP = mybir.AluOpType

L = 4          # layers
NPOS = 512     # 2^9 LUT combos
HW = 1024      # 32*32 grid
BLOC = 2       # batch per core (16 / 8 cores)
NCORES = 8
THV = 5        # th lanes on DVE (th 0..4)
THP = 3        # th lanes on Pool (th 5..7)

MASK_M = [(w - 1) % 32 for w in range(32)]   # j=0: read w-1
MASK_P = [(w + 1) % 32 for w in range(32)]   # j=2: read w+1

_CACHE = {}


def _emit_rolled(nc, eng, dst, src):
    """dst = src rolled by +1 in h (PM layout [128, 16], t = b*8+th)."""
    # ph 0..2 rows: partition shift +32 (quadrant-aligned pieces)
    eng.tensor_copy(out=dst[0:32, :], in_=src[32:64, :])
    eng.tensor_copy(out=dst[32:64, :], in_=src[64:96, :])
    eng.tensor_copy(out=dst[64:96, :], in_=src[96:128, :])
    # ph=3 rows: h+1 lands in th+1 (with th 7 -> 0 wrap within the same b)
    dv = dst[96:128, :].rearrange("p (b th) -> p b th", b=2, th=8)
    sv = src[0:32, :].rearrange("p (b th) -> p b th", b=2, th=8)
    eng.tensor_copy(out=dv[:, :, 0:7], in_=sv[:, :, 1:8])
    eng.tensor_copy(out=dv[:, :, 7:8], in_=sv[:, :, 0:1])


def _tree_slice(nc, eng, a0, cms, state, tho, TH, pool, tag):
    """Emit one engine slice's 9-level tree.  a0: [128, 512*TH] interleaved
    (q*TH + th).  cms[s]: [128,16] multiplier tile for level s.  Writes
    state[:, (b,th) slice] (clipped)."""
    nt = 2 * TH

    def cview(s, q):
        c = cms[s]
        return (
            c[:, :]
            .rearrange("p (b th) -> p b th", b=2, th=8)[:, :, tho : tho + TH]
            .unsqueeze(1)
            .broadcast_to((128, q, 2, TH))
        )

    # ---- level 0: reads shared-over-b a0, writes per-lane A1
    Q = 256
    d0 = pool.tile([128, Q * TH], F16, tag=f"{tag}d0")
    eng.tensor_sub(d0[:, :], a0[:, Q * TH : 2 * Q * TH], a0[:, 0 : Q * TH])
    d0v = (
        d0[:, :]
        .rearrange("p (q th) -> p q th", q=Q, th=TH)
        .unsqueeze(2)
        .broadcast_to((128, Q, 2, TH))
    )
    a0lo = (
        a0[:, 0 : Q * TH]
        .rearrange("p (q th) -> p q th", q=Q, th=TH)
        .unsqueeze(2)
        .broadcast_to((128, Q, 2, TH))
    )
    e0 = pool.tile([128, Q * nt], F16, tag=f"{tag}e0")
    e0v = e0[:, :].rearrange("p (q b th) -> p q b th", q=Q, b=2, th=TH)
    eng.tensor_tensor(out=e0v, in0=d0v, in1=cview(0, Q), op=OP.mult)
    A = pool.tile([128, Q * nt], F16, tag=f"{tag}A1")
    Av = A[:, :].rearrange("p (q b th) -> p q b th", q=Q, b=2, th=TH)
    eng.tensor_tensor(out=Av, in0=e0v, in1=a0lo, op=OP.add)

    # ---- levels 1..8
    for s in range(1, 9):
        Qh = 256 >> s
        d = pool.tile([128, Qh * nt], F16, tag=f"{tag}d{s}")
        eng.tensor_sub(d[:, :], A[:, Qh * nt : 2 * Qh * nt], A[:, 0 : Qh * nt])
        dv = d[:, :].rearrange("p (q b th) -> p q b th", q=Qh, b=2, th=TH)
        e = pool.tile([128, Qh * nt], F16, tag=f"{tag}e{s}")
        ev = e[:, :].rearrange("p (q b th) -> p q b th", q=Qh, b=2, th=TH)
        eng.tensor_tensor(out=ev, in0=dv, in1=cview(s, Qh), op=OP.mult)
        alo = A[:, 0 : Qh * nt].rearrange(
            "p (q b th) -> p q b th", q=Qh, b=2, th=TH
        )
        if s == 8:
            outv = (
                state[:, :]
                .rearrange("p (b th) -> p b th", b=2, th=8)[:, :, tho : tho + TH]
                .unsqueeze(1)
            )
            eng.tensor_tensor(out=outv, in0=ev, in1=alo, op=OP.add)
        else:
            A2 = pool.tile([128, Qh * nt], F16, tag=f"{tag}A{s+1}")
            A2v = A2[:, :].rearrange("p (q b th) -> p q b th", q=Qh, b=2, th=TH)
            eng.tensor_tensor(out=A2v, in0=ev, in1=alo, op=OP.add)
            A = A2
    # clip this slice in place
    stv = state[:, :].rearrange("p (b th) -> p b th", b=2, th=8)[
        :, :, tho : tho + TH
    ]
    eng.tensor_scalar(stv, stv, 0.0, 1.0, OP.max, OP.min)


def _build():
    nc = bacc.Bacc("TRN2", target_bir_lowering=False, debug=True)

    xpm = nc.declare_dram_parameter("xpm", [128, 16], F16, isOutput=False)
    tgh = nc.declare_dram_parameter("tgh", [L, 128, 8 * NPOS], F16, isOutput=False)
    out = nc.declare_dram_parameter("out", [128, 16], F32, isOutput=True)

    with tile.TileContext(nc) as tc:
        with (
            tc.tile_pool(name="tg", bufs=2) as tgp,
            tc.tile_pool(name="a0", bufs=2) as a0p,
            tc.tile_pool(name="st", bufs=2) as stp,
            tc.tile_pool(name="cm", bufs=2) as cmp_,
            tc.tile_pool(name="trv", bufs=1) as trv,
            tc.tile_pool(name="trp", bufs=1) as trp,
        ):
            state = stp.tile([128, 16], F16, tag="state0")
            nc.sync.dma_start(out=state[:, :], in_=xpm[:, :])

            for l in range(L):
                # ---- prefetch + sigmoid (runs during previous layer's tree)
                tgt = tgp.tile([128, 8 * NPOS], F16, tag="tgt")
                nc.sync.dma_start(out=tgt[:, 0 : THV * NPOS], in_=tgh[l, :, 0 : THV * NPOS])
                nc.sync.dma_start(out=tgt[:, THV * NPOS :], in_=tgh[l, :, THV * NPOS :])
                a0v = a0p.tile([128, NPOS * THV], F16, tag="a0v")
                a0q = a0p.tile([128, NPOS * THP], F16, tag="a0q")
                nc.scalar.activation(
                    a0v[:, :].rearrange("p (q th) -> p th q", q=NPOS, th=THV),
                    tgt[:, 0 : THV * NPOS].rearrange("p (th q) -> p th q", th=THV, q=NPOS),
                    AF.Sigmoid,
                )
                nc.scalar.activation(
                    a0q[:, :].rearrange("p (q th) -> p th q", q=NPOS, th=THP),
                    tgt[:, THV * NPOS :].rearrange("p (th q) -> p th q", th=THP, q=NPOS),
                    AF.Sigmoid,
                )

                # ---- window multipliers from state (c_m, m = i*3+j)
                r1 = cmp_.tile([128, 16], F16, tag="r1")
                r2 = cmp_.tile([128, 16], F16, tag="r2")
                cms = [None] * 9
                cms[1] = state   # (0,1): identity
                for m in (0, 2, 3, 5, 6, 8):
                    cms[m] = cmp_.tile([128, 16], F16, tag=f"cm{m}", name=f"cm{m}_t")
                cms[4] = r1      # (1,1)
                cms[7] = r2      # (2,1)
                _emit_rolled(nc, nc.gpsimd, r1, state)
                _emit_rolled(nc, nc.gpsimd, r2, r1)
                nc.vector.stream_shuffle(cms[0][:, :], state[:, :], MASK_M)
                nc.vector.stream_shuffle(cms[2][:, :], state[:, :], MASK_P)
                nc.vector.stream_shuffle(cms[3][:, :], r1[:, :], MASK_M)
                nc.vector.stream_shuffle(cms[5][:, :], r1[:, :], MASK_P)
                nc.vector.stream_shuffle(cms[6][:, :], r2[:, :], MASK_M)
                nc.vector.stream_shuffle(cms[8][:, :], r2[:, :], MASK_P)

                # ---- the two engine tree slices
                newstate = stp.tile([128, 16], F16, tag="state")
                _tree_slice(nc, nc.vector, a0v, cms, newstate, 0, THV, trv, "v")
                _tree_slice(nc, nc.gpsimd, a0q, cms, newstate, THV, THP, trp, "q")
                state = newstate

            outsb = stp.tile([128, 16], F32, tag="outsb")
            nc.vector.tensor_copy(out=outsb[:, :], in_=state[:, :])
            nc.sync.dma_start(out=out[:, :], in_=outsb[:, :])

    nc.finalize()
    return nc


def _host_inputs(x, tg):
    """x: [16,32,32] f32; tg: [4,512,32,32] f32 -> per-core xpm + shared tgh."""
    tgh = np.ascontiguousarray(
        tg.reshape(L, NPOS, 8, 4, 32).transpose(0, 3, 4, 2, 1).reshape(L, 128, 8 * NPOS)
    ).astype(np.float16)
    xpms = []
    for c in range(NCORES):
        xc = x[BLOC * c : BLOC * (c + 1)].reshape(BLOC, 8, 4, 32)
        xpms.append(
            np.ascontiguousarray(xc.transpose(2, 3, 0, 1).reshape(128, 16)).astype(
                np.float16
            )
        )
    return xpms, tgh


def _unpack_out(pm):
    """pm: [128, 16] f32 -> [2, 32, 32]."""
    return np.ascontiguousarray(
        pm.reshape(4, 32, BLOC, 8).transpose(2, 3, 0, 1).reshape(BLOC, 32, 32)
    )


def _run(x, toggle_gates, trace=False):
    if "nc" not in _CACHE:
        _CACHE["nc"] = _build()
    nc = _CACHE["nc"]

    x = np.asarray(x, dtype=np.float32)
    tg = np.asarray(toggle_gates, dtype=np.float32)
    xpms, tgh = _host_inputs(x, tg)
    in_maps = [{"xpm": xpms[c], "tgh": tgh} for c in range(NCORES)]

    res = run_bass_kernel_spmd(nc, in_maps, core_ids=list(range(NCORES)), trace=trace)
    outs = []
    for c in range(NCORES):
        pm = np.asarray(res.results[c]["out"])
        outs.append(_unpack_out(pm))
    full = np.concatenate(outs, axis=0)
    return full, res


def kernel(x, toggle_gates):
    full, _ = _run(x, toggle_gates, trace=False)
    return full


# revision 9
# speedup vs baseline: 4.1838x; 1.0803x over previous
"""Trainium2 Bass kernel for the soft-LUT cellular-ASIC module (fast path).

Math per layer:  state'[b,hw] = clip( sum_p tw[l,p,hw] * prod_m f(c_m, bit_m(p)) )
where c_m[b,hw] = state[b,(h+i)%32,(w+j-1)%32]  (m = i*3+j),  f(c,0)=1-c, f(c,1)=c,
tw = sigmoid(toggle_gates).  bit_m(p) = bit (8-m) of p (m=0 is the MSB).

Implementation: 9-level lerp tree ("soft-LUT contraction"), evaluated in f16
batched across all 16 position tiles at once.  Layout: partition p = ph*32+w
(ph = h%4, w), tile lane t = b*8+th (th = h//4).  The tree state for an engine
slice lives as A[:, q*nt + b*TH + th] (q = remaining LUT combos, th-minor), so
each level is 3 large tensor_tensor ops:
    d = A_hi - A_lo ; e = d * c_bcast ; A' = e + A_lo
with c_bcast a stride-0 broadcast AP over q (keeps the f16 2x DVE mode: the
cost model only requires the innermost AP dim packed).

Window gathers are done on-chip: h-rolls are quadrant-aligned partition copies
(+ th-shift pieces for ph wrap), w-rolls are stream_shuffle ops (within-32
partition permutation).  No DRAM round-trip between layers.

Engine split: DVE owns th 0..4 (10 of 16 lanes), Pool (gpsimd) owns th 5..7;
the two tree chains are fully independent per layer and only join at the
[128,16] state tile.  Act does the sigmoids (strided interleaved writes),
prefetched one layer ahead; toggle gates stream in as f16, one layer per DMA.

Sharding: data-parallel over batch B=16 across 8 cores (B_local=2, no comms).
"""

import numpy as np

import concourse.bass as bass
import concourse.bacc as bacc
import concourse.mybir as mybir
from concourse import tile
from concourse.bass_utils import run_bass_kernel_spmd

F32 = mybir.dt.float32
F16 = mybir.dt.float16
AF = mybir.ActivationFunctionType
OP = mybir.AluOpType

L = 4          # layers
NPOS = 512     # 2^9 LUT combos
HW = 1024      # 32*32 grid
BLOC = 2       # batch per core (16 / 8 cores)
NCORES = 8
THV = 5        # th lanes on DVE (th 0..4)
THP = 3        # th lanes on Pool (th 5..7)

MASK_M = [(w - 1) % 32 for w in range(32)]   # j=0: read w-1
MASK_P = [(w + 1) % 32 for w in range(32)]   # j=2: read w+1

_CACHE = {}


def _emit_rolled(nc, eng, dst, src):
    """dst = src rolled by +1 in h (PM layout [128, 16], t = b*8+th)."""
    # ph 0..2 rows: partition shift +32 (quadrant-aligned pieces)
    eng.tensor_copy(out=dst[0:32, :], in_=src[32:64, :])
    eng.tensor_copy(out=dst[32:64, :], in_=src[64:96, :])
    eng.tensor_copy(out=dst[64:96, :], in_=src[96:128, :])
    # ph=3 rows: h+1 lands in th+1 (with th 7 -> 0 wrap within the same b)
    dv = dst[96:128, :].rearrange("p (b th) -> p b th", b=2, th=8)
    sv = src[0:32, :].rearrange("p (b th) -> p b th", b=2, th=8)
    eng.tensor_copy(out=dv[:, :, 0:7], in_=sv[:, :, 1:8])
    eng.tensor_copy(out=dv[:, :, 7:8], in_=sv[:, :, 0:1])


SPLIT_LEVEL = 5  # levels >= SPLIT_LEVEL run merged on Pool


def _tree_slice(nc, eng, a0, cms, tho, TH, pool, tag):
    """Emit one engine slice's tree, levels 0..SPLIT_LEVEL-1.  a0: [128,
    512*TH] interleaved (q*TH + th).  cms[s]: [128,16] multiplier tile for
    level s.  Returns the [128, Q*2*TH] tile entering level SPLIT_LEVEL."""
    nt = 2 * TH

    def cview(s, q):
        c = cms[s]
        return (
            c[:, :]
            .rearrange("p (b th) -> p b th", b=2, th=8)[:, :, tho : tho + TH]
            .unsqueeze(1)
            .broadcast_to((128, q, 2, TH))
        )

    # ---- level 0: reads shared-over-b a0, writes per-lane A1
    Q = 256
    d0 = pool.tile([128, Q * TH], F16, tag=f"{tag}d0")
    eng.tensor_sub(d0[:, :], a0[:, Q * TH : 2 * Q * TH], a0[:, 0 : Q * TH])
    d0v = (
        d0[:, :]
        .rearrange("p (q th) -> p q th", q=Q, th=TH)
        .unsqueeze(2)
        .broadcast_to((128, Q, 2, TH))
    )
    a0lo = (
        a0[:, 0 : Q * TH]
        .rearrange("p (q th) -> p q th", q=Q, th=TH)
        .unsqueeze(2)
        .broadcast_to((128, Q, 2, TH))
    )
    e0 = pool.tile([128, Q * nt], F16, tag=f"{tag}e0")
    e0v = e0[:, :].rearrange("p (q b th) -> p q b th", q=Q, b=2, th=TH)
    eng.tensor_tensor(out=e0v, in0=d0v, in1=cview(0, Q), op=OP.mult)
    A = pool.tile([128, Q * nt], F16, tag=f"{tag}A1")
    Av = A[:, :].rearrange("p (q b th) -> p q b th", q=Q, b=2, th=TH)
    eng.tensor_tensor(out=Av, in0=e0v, in1=a0lo, op=OP.add)

    # ---- levels 1..SPLIT_LEVEL-1
    for s in range(1, SPLIT_LEVEL):
        Qh = 256 >> s
        d = pool.tile([128, Qh * nt], F16, tag=f"{tag}d{s}")
        eng.tensor_sub(d[:, :], A[:, Qh * nt : 2 * Qh * nt], A[:, 0 : Qh * nt])
        dv = d[:, :].rearrange("p (q b th) -> p q b th", q=Qh, b=2, th=TH)
        e = pool.tile([128, Qh * nt], F16, tag=f"{tag}e{s}")
        ev = e[:, :].rearrange("p (q b th) -> p q b th", q=Qh, b=2, th=TH)
        eng.tensor_tensor(out=ev, in0=dv, in1=cview(s, Qh), op=OP.mult)
        alo = A[:, 0 : Qh * nt].rearrange(
            "p (q b th) -> p q b th", q=Qh, b=2, th=TH
        )
        A2 = pool.tile([128, Qh * nt], F16, tag=f"{tag}A{s+1}")
        A2v = A2[:, :].rearrange("p (q b th) -> p q b th", q=Qh, b=2, th=TH)
        eng.tensor_tensor(out=A2v, in0=ev, in1=alo, op=OP.add)
        A = A2
    return A


def _tree_tail(nc, eng, Av, Ap, cms, state, pool):
    """Merge the two engine slices at SPLIT_LEVEL and finish levels
    SPLIT_LEVEL..8 full-width on one engine; clip into state [128,16]."""
    Q = 256 >> (SPLIT_LEVEL - 1)  # combos entering SPLIT_LEVEL
    Am = pool.tile([128, Q * 16], F16, tag="Am")
    Amv = Am[:, :].rearrange("p (q b th) -> p q b th", q=Q, b=2, th=8)
    eng.tensor_copy(
        out=Amv[:, :, :, 0:THV],
        in_=Av[:, :].rearrange("p (q b th) -> p q b th", q=Q, b=2, th=THV),
    )
    eng.tensor_copy(
        out=Amv[:, :, :, THV:8],
        in_=Ap[:, :].rearrange("p (q b th) -> p q b th", q=Q, b=2, th=THP),
    )
    A = Am
    for s in range(SPLIT_LEVEL, 9):
        Qh = 256 >> s
        d = pool.tile([128, Qh * 16], F16, tag=f"md{s}")
        eng.tensor_sub(d[:, :], A[:, Qh * 16 : 2 * Qh * 16], A[:, 0 : Qh * 16])
        dv = d[:, :].rearrange("p (q t) -> p q t", q=Qh, t=16)
        cv = cms[s][:, :].unsqueeze(1).broadcast_to((128, Qh, 16))
        e = pool.tile([128, Qh * 16], F16, tag=f"me{s}")
        ev = e[:, :].rearrange("p (q t) -> p q t", q=Qh, t=16)
        eng.tensor_tensor(out=ev, in0=dv, in1=cv, op=OP.mult)
        alo = A[:, 0 : Qh * 16].rearrange("p (q t) -> p q t", q=Qh, t=16)
        if s == 8:
            outv = state[:, :].unsqueeze(1)
            eng.tensor_tensor(out=outv, in0=ev, in1=alo, op=OP.add)
        else:
            A2 = pool.tile([128, Qh * 16], F16, tag=f"mA{s+1}")
            A2v = A2[:, :].rearrange("p (q t) -> p q t", q=Qh, t=16)
            eng.tensor_tensor(out=A2v, in0=ev, in1=alo, op=OP.add)
            A = A2
    eng.tensor_scalar(state[:, :], state[:, :], 0.0, 1.0, OP.max, OP.min)


def _build():
    nc = bacc.Bacc("TRN2", target_bir_lowering=False, debug=True)

    xpm = nc.declare_dram_parameter("xpm", [128, 16], F16, isOutput=False)
    # a0h: layer-0 LUT table pre-activated host-side (interleaved layout),
    # layers 1..3 stream in raw and are activated on-chip during the
    # previous layer's tree.
    a0h = nc.declare_dram_parameter("a0h", [128, 8 * NPOS], F16, isOutput=False)
    tgh = nc.declare_dram_parameter("tgh", [L, 128, 8 * NPOS], F16, isOutput=False)
    out = nc.declare_dram_parameter("out", [128, 16], F32, isOutput=True)

    with tile.TileContext(nc) as tc:
        with (
            tc.tile_pool(name="tg", bufs=2) as tgp,
            tc.tile_pool(name="a0", bufs=2) as a0p,
            tc.tile_pool(name="st", bufs=2) as stp,
            tc.tile_pool(name="cm", bufs=2) as cmp_,
            tc.tile_pool(name="trv", bufs=1) as trv,
            tc.tile_pool(name="trp", bufs=1) as trp,
        ):
            state = stp.tile([128, 16], F16, tag="state0")
            nc.sync.dma_start(out=state[:, :], in_=xpm[:, :])

            for l in range(L):
                # ---- prefetch + sigmoid (runs during previous layer's tree)
                a0v = a0p.tile([128, NPOS * THV], F16, tag="a0v")
                a0q = a0p.tile([128, NPOS * THP], F16, tag="a0q")
                if l == 0:
                    nc.sync.dma_start(out=a0v[:, :], in_=a0h[:, 0 : THV * NPOS])
                    nc.sync.dma_start(out=a0q[:, :], in_=a0h[:, THV * NPOS :])
                else:
                    tgt = tgp.tile([128, 8 * NPOS], F16, tag="tgt")
                    nc.sync.dma_start(
                        out=tgt[:, 0 : THV * NPOS], in_=tgh[l, :, 0 : THV * NPOS]
                    )
                    nc.sync.dma_start(
                        out=tgt[:, THV * NPOS :], in_=tgh[l, :, THV * NPOS :]
                    )
                    nc.scalar.activation(
                        a0v[:, :].rearrange("p (q th) -> p th q", q=NPOS, th=THV),
                        tgt[:, 0 : THV * NPOS].rearrange(
                            "p (th q) -> p th q", th=THV, q=NPOS
                        ),
                        AF.Sigmoid,
                    )
                    nc.scalar.activation(
                        a0q[:, :].rearrange("p (q th) -> p th q", q=NPOS, th=THP),
                        tgt[:, THV * NPOS :].rearrange(
                            "p (th q) -> p th q", th=THP, q=NPOS
                        ),
                        AF.Sigmoid,
                    )

                # ---- window multipliers from state (c_m, m = i*3+j)
                r1 = cmp_.tile([128, 16], F16, tag="r1")
                r2 = cmp_.tile([128, 16], F16, tag="r2")
                cms = [None] * 9
                cms[1] = state   # (0,1): identity
                for m in (0, 2, 3, 5, 6, 8):
                    cms[m] = cmp_.tile([128, 16], F16, tag=f"cm{m}", name=f"cm{m}_t")
                cms[4] = r1      # (1,1)
                cms[7] = r2      # (2,1)
                _emit_rolled(nc, nc.gpsimd, r1, state)
                _emit_rolled(nc, nc.gpsimd, r2, r1)
                nc.vector.stream_shuffle(cms[0][:, :], state[:, :], MASK_M)
                nc.vector.stream_shuffle(cms[2][:, :], state[:, :], MASK_P)
                nc.vector.stream_shuffle(cms[3][:, :], r1[:, :], MASK_M)
                nc.vector.stream_shuffle(cms[5][:, :], r1[:, :], MASK_P)
                nc.vector.stream_shuffle(cms[6][:, :], r2[:, :], MASK_M)
                nc.vector.stream_shuffle(cms[8][:, :], r2[:, :], MASK_P)

                # ---- the two engine tree slices + merged tail on Pool
                newstate = stp.tile([128, 16], F16, tag="state")
                Av = _tree_slice(nc, nc.vector, a0v, cms, 0, THV, trv, "v")
                Ap = _tree_slice(nc, nc.gpsimd, a0q, cms, THV, THP, trp, "q")
                _tree_tail(nc, nc.gpsimd, Av, Ap, cms, newstate, trp)
                state = newstate

            outsb = stp.tile([128, 16], F32, tag="outsb")
            nc.gpsimd.tensor_copy(out=outsb[:, :], in_=state[:, :])
            nc.sync.dma_start(out=out[:, :], in_=outsb[:, :])

    nc.finalize()
    return nc


def _host_inputs(x, tg):
    """x: [16,32,32] f32; tg: [4,512,32,32] f32 -> per-core xpm + shared
    tgh/a0h.  tgh[l, p, th*512+q]; a0h[p, :2560] = sig(l0)[q*5+th (th<5)],
    a0h[p, 2560:] = sig(l0)[q*3+(th-5)]."""
    tgq = tg.reshape(L, NPOS, 8, 4, 32).transpose(0, 3, 4, 2, 1)  # l, ph, w, th, q
    tgh = np.ascontiguousarray(tgq.reshape(L, 128, 8 * NPOS)).astype(np.float16)
    sig0 = 1.0 / (1.0 + np.exp(-tgq[0].reshape(128, 8, NPOS).astype(np.float32)))
    sig0 = sig0.astype(np.float16)  # [p, th, q]
    a0h = np.concatenate(
        [
            sig0[:, 0:THV, :].transpose(0, 2, 1).reshape(128, NPOS * THV),
            sig0[:, THV:8, :].transpose(0, 2, 1).reshape(128, NPOS * THP),
        ],
        axis=1,
    )
    a0h = np.ascontiguousarray(a0h)
    xpms = []
    for c in range(NCORES):
        xc = x[BLOC * c : BLOC * (c + 1)].reshape(BLOC, 8, 4, 32)
        xpms.append(
            np.ascontiguousarray(xc.transpose(2, 3, 0, 1).reshape(128, 16)).astype(
                np.float16
            )
        )
    return xpms, tgh, a0h


def _unpack_out(pm):
    """pm: [128, 16] f32 -> [2, 32, 32]."""
    return np.ascontiguousarray(
        pm.reshape(4, 32, BLOC, 8).transpose(2, 3, 0, 1).reshape(BLOC, 32, 32)
    )


def _run(x, toggle_gates, trace=False):
    if "nc" not in _CACHE:
        _CACHE["nc"] = _build()
    nc = _CACHE["nc"]

    x = np.asarray(x, dtype=np.float32)
    tg = np.asarray(toggle_gates, dtype=np.float32)
    xpms, tgh, a0h = _host_inputs(x, tg)
    in_maps = [{"xpm": xpms[c], "tgh": tgh, "a0h": a0h} for c in range(NCORES)]

    res = run_bass_kernel_spmd(nc, in_maps, core_ids=list(range(NCORES)), trace=trace)
    outs = []
    for c in range(NCORES):
        pm = np.asarray(res.results[c]["out"])
        outs.append(_unpack_out(pm))
    full = np.concatenate(outs, axis=0)
    return full, res


def kernel(x, toggle_gates):
    full, _ = _run(x, toggle_gates, trace=False)
    return full
